# revision 42
# baseline (speedup 1.0000x reference)
"""Bahdanau-style attention layer on 8 Trainium2 NeuronCores.

Math (per batch b):
    bias  = dec[b] @ W2                              [D]
    score = tanh(enc[b] @ W1 + bias)                 [T, D]
    logit = score @ V                                [T]
    w     = softmax(logit)  (over T)                 [T]
    ctx   = sum_t w[t] * enc[b, t]                   [D]
Returns (ctx [B, D] f32, w [B, T, 1] f32).

Sharding: data-parallel over batch, 4 batches per core, W1/W2/V replicated.

Per-core kernel design (T=8192, D=128), one NeuronCore per 4 batches:
  * enc is cast f32->fp16 in-flight during the HBM load (SWDGE cast DMA)
    and kept SBUF-resident per batch in natural [t=128, chunk, d=128]
    layout. HBM is read exactly once (~16.8 MB/core, the roofline term).
  * encT (d on partitions) for the score matmul is produced by PE-mode
    transposes (fp16 stationary x identity -> fp16 PSUM), evacuated to
    SBUF by DVE copies viewed as int32 (half the element count).
  * scoreT chunk [e=128, 1024] = W1^T @ encT on PE (fp16 in, f32 psum).
  * tanh(score + bias) on ACT in 1024-wide chunks (bias per-partition AP).
  * logits: V is zero-padded to 32 columns so each [32, 512] matmul fills
    a full partition group; chunk c = 4g+j lands in row 32j of PSUM bank
    g via tile_position, so one contiguous ACT exp evacuates 4 chunks
    (rows != 32j hold exp(0) = 1 and are masked out of the sum).
  * no max-subtraction in softmax: |logit| <= sum|V| ~ 8, exp is safely
    in range. exp row-sums come free via the ACT accum_out.
  * ctx accumulates with UNnormalized fp16 exp-weights (within fp16
    normal range) over 64 N=2 matmuls per batch (enc chunk stationary),
    so it pipelines without waiting for the softmax sum; 1/sum is applied
    at the [128, 1] PSUM evacuation. Weight columns [t=128, 1] come from
    PE transposes of the exp rows.
  * batch b's softmax/ctx tail is emitted after batch b+1's first score
    chunk so PE always has independent work during the reduction chain.
"""

import numpy as np

B, T, D, H = 32, 8192, 128, 128
N_CORES = 8
B_LOC = B // N_CORES          # 4 batches per core
CH = 512                      # t elements per score/logit chunk
N_CH = T // CH                # 16 chunks per batch
N_TILE = T // 128             # 64 natural [128, 128] tiles per batch

_BUILT = None


def _build():
    from contextlib import ExitStack

    import concourse.bass as bass
    import concourse.mybir as mybir
    import concourse.tile as tile
    from concourse import bacc
    from concourse.masks import make_identity

    f32 = mybir.dt.float32
    f16 = mybir.dt.float16
    AF = mybir.ActivationFunctionType

    nc = bacc.Bacc("TRN2", target_bir_lowering=False)

    enc = nc.dram_tensor("enc", [B_LOC, T, D], f32, kind="ExternalInput")
    dec = nc.dram_tensor("dec", [B_LOC, H], f32, kind="ExternalInput")
    w1 = nc.dram_tensor("w1", [D, D], f32, kind="ExternalInput")
    w2 = nc.dram_tensor("w2", [H, D], f32, kind="ExternalInput")
    vv = nc.dram_tensor("v", [D, 1], f32, kind="ExternalInput")
    ctx_out = nc.dram_tensor("ctx_out", [B_LOC, D], f32, kind="ExternalOutput")
    attn_out = nc.dram_tensor("attn_out", [B_LOC, T], f32, kind="ExternalOutput")

    with tile.TileContext(nc) as tc, ExitStack() as ctx:
        ep = ctx.enter_context  # shorthand

        # ---- pools ----
        p_const = ep(tc.tile_pool(name="const", bufs=1))
        p_enc = ep(tc.tile_pool(name="enc", bufs=3))
        p_encT = ep(tc.tile_pool(name="encT", bufs=2))
        p_tanh = ep(tc.tile_pool(name="tanh", bufs=2))
        p_small = ep(tc.tile_pool(name="small", bufs=2))
        p_ps_score = ep(tc.tile_pool(name="ps_score", bufs=2, space="PSUM"))
        p_ps_lg = ep(tc.tile_pool(name="ps_lg", bufs=1, space="PSUM"))
        p_ps_ctx = ep(tc.tile_pool(name="ps_ctx", bufs=1, space="PSUM"))
        p_ps_tr = ep(tc.tile_pool(name="ps_tr", bufs=2, space="PSUM"))

        # ---- one-time setup ----
        w1_h = p_const.tile([128, 128], f16)
        nc.gpsimd.dma_start(w1_h[:], w1.ap())          # cast f32->fp16 in DMA
        w2_h = p_const.tile([128, 128], f16)
        nc.gpsimd.dma_start(w2_h[:], w2.ap())
        v_h = p_const.tile([128, 1], f16)
        nc.gpsimd.dma_start(v_h[:], vv.ap())
        decT_h = p_const.tile([128, B_LOC], f16)
        nc.gpsimd.dma_start(decT_h[:], dec.ap().rearrange("b h -> h b"))
        ones_row = p_const.tile([1, 128], f32)
        nc.vector.memset(ones_row[:], 1.0)
        # V padded to 32 columns of zeros so the logits matmul fills a full
        # 32-partition group (only row 32j of each group is meaningful).
        v_pad = p_const.tile([128, 32], f16)
        nc.vector.memset(v_pad[:], 0.0)
        nc.vector.tensor_copy(v_pad[:, 0:1], v_h[:])
        # fp16 identity for PE-mode transposes
        ident_h = p_const.tile([128, 128], f16)
        make_identity(nc, ident_h[:])
        # mask selecting rows {0, 32, 64, 96} (where real logits live)
        mask = p_const.tile([128, 1], f32)
        nc.vector.memset(mask[:], 0.0)
        for j in range(4):
            nc.vector.memset(mask[32 * j : 32 * j + 1, :], 1.0)

        # biasT[d, b] = sum_h W2[h, d] * dec[b, h]
        biasT_ps = p_ps_score.tile([128, B_LOC], f32, tag="score")
        nc.tensor.matmul(biasT_ps[:], w2_h[:], decT_h[:], start=True, stop=True)
        biasT_sb = p_const.tile([128, B_LOC], f32)
        nc.scalar.activation(biasT_sb[:], biasT_ps[:], AF.Copy)

        # ctx uses N=2 matmuls (a garbage second column) because N=1 moving
        # APs collapse to 1-D, which the BIR verifier rejects. The same bank
        # also hosts the per-batch softmax scalars (cols 8+).
        ctx_ps = p_ps_ctx.tile([128, 2 * B_LOC + 8 * B_LOC], f32)
        ctx_sb = p_const.tile([128, B_LOC], f32)

        pending_tail = None

        for b in range(B_LOC):
            # ---- load + transpose enc (fp16) ----
            enc_nat = p_enc.tile([128, N_TILE, 128], f16, tag="enc")
            src = enc.ap()[b].rearrange("(n p) d -> p n d", p=128)
            for q in range(8):
                nc.gpsimd.dma_start(
                    enc_nat[:, 8 * q : 8 * (q + 1), :],
                    src[:, 8 * q : 8 * (q + 1), :],
                )
            encT = p_encT.tile([128, N_TILE, 128], f16, tag="encT")

            # ---- transpose (PE) + scores + tanh ----
            tanh_sb = p_tanh.tile([128, N_CH, CH], f16, tag="tanh")
            i32 = mybir.dt.int32
            # Logits chunk c = 4g + j lands in row 32j of PSUM bank g (rows
            # 32j+1..32j+31 get zeros from the padded V columns), so each
            # bank is fully written and evacuates with one contiguous ACT
            # exp op. Logit groups are interleaved into the score loop as
            # soon as their tanh inputs exist.
            exp_full = p_small.tile([128, 4 * CH], f32, tag="exp")
            rs4 = p_small.tile([128, 4], f32, tag="rs4")

            w16 = p_small.tile([128, 4 * CH], f16, tag="w16")
            wT = p_small.tile([128, 16, 128], f16, tag="wT")

            def emit_logits_group(g):
                lg = p_ps_lg.tile([128, CH], f32, tag="lg")
                for j in range(4):
                    c = 4 * g + j
                    nc.tensor.matmul(
                        lg[32 * j : 32 * j + 32, :],
                        v_pad[:],
                        tanh_sb[:, c, :],
                        start=True,
                        stop=True,
                        tile_position=(0, 32 * j),
                    )
                nc.scalar.activation(
                    exp_full[:, CH * g : CH * (g + 1)], lg[:], AF.Exp,
                    accum_out=rs4[:, g : g + 1],
                )
                # Unnormalized fp16 weights (exp(logit) is within fp16 normal
                # range since |logit| <= sum|V|); 1/sum is applied at ctx evac,
                # so the ctx matmuls don't wait on the full softmax sum.
                nc.vector.tensor_copy(
                    w16[:, CH * g : CH * (g + 1)],
                    exp_full[:, CH * g : CH * (g + 1)],
                )

            def emit_ctx_group(g, b=b, w16=w16, wT=wT, enc_nat=enc_nat):
                tpw = p_ps_tr.tile([128, CH], f16, tag="tr")
                for mq in range(4):
                    m = 4 * g + mq
                    nc.tensor.transpose(
                        tpw[:, 128 * mq : 128 * mq + 128],
                        w16[:, 128 * m : 128 * (m + 1)],
                        ident_h[:],
                    )
                nc.vector.tensor_copy(
                    wT[:, 4 * g : 4 * (g + 1), :].bitcast(i32),
                    tpw[:].bitcast(i32),
                )
                for k in range(16 * g, 16 * g + 16):
                    r = 32 * ((k // 4) % 4)
                    nc.tensor.matmul(
                        ctx_ps[:, 2 * b : 2 * b + 2],
                        enc_nat[:, k, :],
                        wT[:, 4 * (k // 16) + k % 4, r : r + 2],
                        start=(k == 0),
                        stop=(k == N_TILE - 1),
                        skip_group_check=True,
                    )

            for cc in range(N_CH // 2):
                sp = p_ps_score.tile([128, 2 * CH], f32, tag="score")
                for c in (2 * cc, 2 * cc + 1):
                    tp = p_ps_tr.tile([128, CH], f16, tag="tr")
                    for q in range(4):
                        k = 4 * c + q
                        nc.tensor.transpose(
                            tp[:, 128 * q : 128 * (q + 1)],
                            enc_nat[:, k, :],
                            ident_h[:],
                        )
                    nc.vector.tensor_copy(
                        encT[:, 4 * c : 4 * (c + 1), :].bitcast(i32),
                        tp[:].bitcast(i32),
                    )
                    nc.tensor.matmul(
                        sp[:, CH * (c - 2 * cc) : CH * (c - 2 * cc + 1)],
                        w1_h[:],
                        encT[:, 4 * c : 4 * (c + 1), :],
                        start=True, stop=True,
                    )
                nc.scalar.activation(
                    tanh_sb[:, 2 * cc : 2 * cc + 2, :], sp[:], AF.Tanh,
                    bias=biasT_sb[:, b : b + 1],
                )
                if cc % 2 == 1:
                    emit_logits_group(cc // 2)
                    if cc // 2 > 0:
                        emit_ctx_group(cc // 2 - 1)
                # the previous batch's softmax/ctx tail is emitted after this
                # batch's first score chunk so PE has work while the tail's
                # exp/sum chain completes on ACT/DVE
                if cc == 0 and pending_tail is not None:
                    pending_tail()
                    pending_tail = None

            def emit_tail(
                b=b, rs4=rs4, exp_full=exp_full, emit_ctx_group=emit_ctx_group
            ):
                emit_ctx_group(3)
                # ---- softmax normalization ----
                # Row 32j of exp_full holds real exp(logits); other rows
                # hold exp(0) = 1 and are excluded from the sum by `mask`.
                sm_ps = ctx_ps[:, 8 + 8 * b : 8 + 8 * b + 8]
                s_row = sm_ps[0:1, 0:4]
                nc.tensor.matmul(s_row, mask[:], rs4[:], start=True, stop=True)
                stot2 = p_small.tile([1, 2], f32, tag="stot")
                nc.vector.memset(stot2[:, 1:2], 1.0)
                nc.vector.tensor_reduce(
                    stot2[:, 0:1], s_row[:], axis=mybir.AxisListType.X,
                    op=mybir.AluOpType.add,
                )
                srec2 = p_small.tile([1, 2], f32, tag="srec")
                nc.vector.reciprocal(srec2[:], stot2[:])
                sc_ps = sm_ps[:, 4:6]
                nc.tensor.matmul(
                    sc_ps, ones_row[:], srec2[:], start=True, stop=True
                )
                scale = p_small.tile([128, 1], f32, tag="scale")
                nc.scalar.activation(scale[:], sc_ps[:, 0:1], AF.Copy)

                # normalized attention weights -> HBM (f32, exact path)
                attn_sb = p_small.tile([128, 4 * CH], f32, tag="attn")
                nc.vector.tensor_scalar_mul(attn_sb[:], exp_full[:], scale[:])
                for g in range(N_CH // 4):
                    nc.sync.dma_start(
                        attn_out.ap()[b, 2048 * g : 2048 * (g + 1)].rearrange(
                            "(j u) -> j u", u=CH
                        ),
                        attn_sb[0:97:32, CH * g : CH * (g + 1)],
                    )

                # ctx accumulated with unnormalized weights; apply 1/sum here
                nc.scalar.activation(
                    ctx_sb[:, b : b + 1], ctx_ps[:, 2 * b : 2 * b + 1], AF.Copy,
                    scale=scale[:],
                )

            pending_tail = emit_tail

        pending_tail()
        nc.sync.dma_start(ctx_out.ap().rearrange("b d -> d b"), ctx_sb[:])

    nc.compile()
    return nc


def _get_nc():
    global _BUILT
    if _BUILT is None:
        _BUILT = _build()
    return _BUILT


def kernel(encoder_outputs, decoder_hidden, W1, W2, V):
    from concourse.bass_utils import run_bass_kernel_spmd

    enc = np.ascontiguousarray(np.asarray(encoder_outputs, dtype=np.float32))
    dec = np.ascontiguousarray(np.asarray(decoder_hidden, dtype=np.float32))
    w1 = np.ascontiguousarray(np.asarray(W1, dtype=np.float32))
    w2 = np.ascontiguousarray(np.asarray(W2, dtype=np.float32))
    v = np.ascontiguousarray(np.asarray(V, dtype=np.float32))

    nc = _get_nc()
    in_maps = []
    for c in range(N_CORES):
        sl = slice(c * B_LOC, (c + 1) * B_LOC)
        in_maps.append(
            {"enc": enc[sl], "dec": dec[sl], "w1": w1, "w2": w2, "v": v}
        )
    res = run_bass_kernel_spmd(nc, in_maps, core_ids=list(range(N_CORES)))
    ctxs = np.concatenate([r["ctx_out"] for r in res.results], axis=0)
    attns = np.concatenate([r["attn_out"] for r in res.results], axis=0)
    return ctxs.astype(np.float32), attns.astype(np.float32)[:, :, None]


# revision 43
# speedup vs baseline: 1.0404x; 1.0404x over previous
"""Bahdanau-style attention layer on 8 Trainium2 NeuronCores.

Math (per batch b):
    bias  = dec[b] @ W2                              [D]
    score = tanh(enc[b] @ W1 + bias)                 [T, D]
    logit = score @ V                                [T]
    w     = softmax(logit)  (over T)                 [T]
    ctx   = sum_t w[t] * enc[b, t]                   [D]
Returns (ctx [B, D] f32, w [B, T, 1] f32).

Sharding: data-parallel over batch, 4 batches per core, W1/W2/V replicated.

Per-core kernel design (T=8192, D=128), one NeuronCore per 4 batches:
  * enc is cast f32->fp16 in-flight during the HBM load (SWDGE cast DMA)
    and kept SBUF-resident per batch in natural [t=128, chunk, d=128]
    layout. HBM is read exactly once (~16.8 MB/core, the roofline term).
  * encT (d on partitions) for the score matmul is produced by PE-mode
    transposes (fp16 stationary x identity -> fp16 PSUM), evacuated to
    SBUF by DVE copies viewed as int32 (half the element count).
  * scoreT chunk [e=128, 1024] = W1^T @ encT on PE (fp16 in, f32 psum).
  * tanh(score + bias) on ACT in 1024-wide chunks (bias per-partition AP).
  * logits: V is zero-padded to 32 columns so each [32, 512] matmul fills
    a full partition group; chunk c = 4g+j lands in row 32j of PSUM bank
    g via tile_position, so one contiguous ACT exp evacuates 4 chunks
    (rows != 32j hold exp(0) = 1 and are masked out of the sum).
  * no max-subtraction in softmax: |logit| <= sum|V| ~ 8, exp is safely
    in range. exp row-sums come free via the ACT accum_out.
  * ctx accumulates with UNnormalized fp16 exp-weights (within fp16
    normal range) over 64 N=2 matmuls per batch (enc chunk stationary),
    so it pipelines without waiting for the softmax sum; 1/sum is applied
    at the [128, 1] PSUM evacuation. Weight columns [t=128, 1] come from
    PE transposes of the exp rows.
  * batch b's softmax/ctx tail is emitted after batch b+1's first score
    chunk so PE always has independent work during the reduction chain.
"""

import numpy as np

B, T, D, H = 32, 8192, 128, 128
N_CORES = 8
B_LOC = B // N_CORES          # 4 batches per core
CH = 512                      # t elements per score/logit chunk
N_CH = T // CH                # 16 chunks per batch
N_TILE = T // 128             # 64 natural [128, 128] tiles per batch

_BUILT = None


def _build():
    from contextlib import ExitStack

    import concourse.bass as bass
    import concourse.mybir as mybir
    import concourse.tile as tile
    from concourse import bacc
    from concourse.masks import make_identity

    f32 = mybir.dt.float32
    f16 = mybir.dt.float16
    AF = mybir.ActivationFunctionType

    nc = bacc.Bacc("TRN2", target_bir_lowering=False)

    enc = nc.dram_tensor("enc", [B_LOC, T, D], f32, kind="ExternalInput")
    dec = nc.dram_tensor("dec", [B_LOC, H], f32, kind="ExternalInput")
    w1 = nc.dram_tensor("w1", [D, D], f32, kind="ExternalInput")
    w2 = nc.dram_tensor("w2", [H, D], f32, kind="ExternalInput")
    vv = nc.dram_tensor("v", [D, 1], f32, kind="ExternalInput")
    ctx_out = nc.dram_tensor("ctx_out", [B_LOC, D], f32, kind="ExternalOutput")
    attn_out = nc.dram_tensor("attn_out", [B_LOC, T], f32, kind="ExternalOutput")

    with tile.TileContext(nc) as tc, ExitStack() as ctx:
        ep = ctx.enter_context  # shorthand

        # ---- pools ----
        p_const = ep(tc.tile_pool(name="const", bufs=1))
        p_enc = ep(tc.tile_pool(name="enc", bufs=3))
        p_encT = ep(tc.tile_pool(name="encT", bufs=2))
        p_tanh = ep(tc.tile_pool(name="tanh", bufs=2))
        p_small = ep(tc.tile_pool(name="small", bufs=2))
        p_ps_score = ep(tc.tile_pool(name="ps_score", bufs=3, space="PSUM"))
        p_ps_lg = ep(tc.tile_pool(name="ps_lg", bufs=1, space="PSUM"))
        p_ps_ctx = ep(tc.tile_pool(name="ps_ctx", bufs=1, space="PSUM"))
        p_ps_tr = ep(tc.tile_pool(name="ps_tr", bufs=3, space="PSUM"))

        # ---- one-time setup ----
        w1_h = p_const.tile([128, 128], f16)
        nc.gpsimd.dma_start(w1_h[:], w1.ap())          # cast f32->fp16 in DMA
        w2_h = p_const.tile([128, 128], f16)
        nc.gpsimd.dma_start(w2_h[:], w2.ap())
        v_h = p_const.tile([128, 1], f16)
        nc.gpsimd.dma_start(v_h[:], vv.ap())
        decT_h = p_const.tile([128, B_LOC], f16)
        nc.gpsimd.dma_start(decT_h[:], dec.ap().rearrange("b h -> h b"))
        ones_row = p_const.tile([1, 128], f32)
        nc.vector.memset(ones_row[:], 1.0)
        # V padded to 32 columns of zeros so the logits matmul fills a full
        # 32-partition group (only row 32j of each group is meaningful).
        v_pad = p_const.tile([128, 32], f16)
        nc.vector.memset(v_pad[:], 0.0)
        nc.vector.tensor_copy(v_pad[:, 0:1], v_h[:])
        # fp16 identity for PE-mode transposes
        ident_h = p_const.tile([128, 128], f16)
        make_identity(nc, ident_h[:])
        # mask selecting rows {0, 32, 64, 96} (where real logits live)
        mask = p_const.tile([128, 1], f32)
        nc.vector.memset(mask[:], 0.0)
        for j in range(4):
            nc.vector.memset(mask[32 * j : 32 * j + 1, :], 1.0)

        # biasT[d, b] = sum_h W2[h, d] * dec[b, h]
        biasT_ps = p_ps_score.tile([128, B_LOC], f32, tag="score")
        nc.tensor.matmul(biasT_ps[:], w2_h[:], decT_h[:], start=True, stop=True)
        biasT_sb = p_const.tile([128, B_LOC], f32)
        nc.scalar.activation(biasT_sb[:], biasT_ps[:], AF.Copy)

        # ctx uses N=2 matmuls (a garbage second column) because N=1 moving
        # APs collapse to 1-D, which the BIR verifier rejects. The same bank
        # also hosts the per-batch softmax scalars (cols 8+).
        ctx_ps = p_ps_ctx.tile([128, 2 * B_LOC + 8 * B_LOC], f32)
        ctx_sb = p_const.tile([128, B_LOC], f32)

        pending_tail = None

        for b in range(B_LOC):
            # ---- load + transpose enc (fp16) ----
            enc_nat = p_enc.tile([128, N_TILE, 128], f16, tag="enc")
            src = enc.ap()[b].rearrange("(n p) d -> p n d", p=128)
            for q in range(8):
                nc.gpsimd.dma_start(
                    enc_nat[:, 8 * q : 8 * (q + 1), :],
                    src[:, 8 * q : 8 * (q + 1), :],
                )
            encT = p_encT.tile([128, N_TILE, 128], f16, tag="encT")

            # ---- transpose (PE) + scores + tanh ----
            tanh_sb = p_tanh.tile([128, N_CH, CH], f16, tag="tanh")
            i32 = mybir.dt.int32
            # Logits chunk c = 4g + j lands in row 32j of PSUM bank g (rows
            # 32j+1..32j+31 get zeros from the padded V columns), so each
            # bank is fully written and evacuates with one contiguous ACT
            # exp op. Logit groups are interleaved into the score loop as
            # soon as their tanh inputs exist.
            exp_full = p_small.tile([128, 4 * CH], f32, tag="exp")
            rs4 = p_small.tile([128, 4], f32, tag="rs4")

            w16 = p_small.tile([128, 4 * CH], f16, tag="w16")
            wT = p_small.tile([128, 16, 128], f16, tag="wT")

            def emit_logits_group(g):
                lg = p_ps_lg.tile([128, CH], f32, tag="lg")
                for j in range(4):
                    c = 4 * g + j
                    nc.tensor.matmul(
                        lg[32 * j : 32 * j + 32, :],
                        v_pad[:],
                        tanh_sb[:, c, :],
                        start=True,
                        stop=True,
                        tile_position=(0, 32 * j),
                    )
                nc.scalar.activation(
                    exp_full[:, CH * g : CH * (g + 1)], lg[:], AF.Exp,
                    accum_out=rs4[:, g : g + 1],
                )
                # Unnormalized fp16 weights (exp(logit) is within fp16 normal
                # range since |logit| <= sum|V|); 1/sum is applied at ctx evac,
                # so the ctx matmuls don't wait on the full softmax sum.
                nc.vector.tensor_copy(
                    w16[:, CH * g : CH * (g + 1)],
                    exp_full[:, CH * g : CH * (g + 1)],
                )

            def emit_ctx_group(g, b=b, w16=w16, wT=wT, enc_nat=enc_nat):
                tpw = p_ps_tr.tile([128, CH], f16, tag="tr")
                for mq in range(4):
                    m = 4 * g + mq
                    nc.tensor.transpose(
                        tpw[:, 128 * mq : 128 * mq + 128],
                        w16[:, 128 * m : 128 * (m + 1)],
                        ident_h[:],
                    )
                nc.vector.tensor_copy(
                    wT[:, 4 * g : 4 * (g + 1), :].bitcast(i32),
                    tpw[:].bitcast(i32),
                )
                for k in range(16 * g, 16 * g + 16):
                    r = 32 * ((k // 4) % 4)
                    nc.tensor.matmul(
                        ctx_ps[:, 2 * b : 2 * b + 2],
                        enc_nat[:, k, :],
                        wT[:, 4 * (k // 16) + k % 4, r : r + 2],
                        start=(k == 0),
                        stop=(k == N_TILE - 1),
                        skip_group_check=True,
                    )

            for cc in range(N_CH // 2):
                for c in (2 * cc, 2 * cc + 1):
                    tp = p_ps_tr.tile([128, CH], f16, tag="tr")
                    for q in range(4):
                        k = 4 * c + q
                        nc.tensor.transpose(
                            tp[:, 128 * q : 128 * (q + 1)],
                            enc_nat[:, k, :],
                            ident_h[:],
                        )
                    nc.vector.tensor_copy(
                        encT[:, 4 * c : 4 * (c + 1), :].bitcast(i32),
                        tp[:].bitcast(i32),
                    )
                    sp = p_ps_score.tile([128, CH], f32, tag="score")
                    nc.tensor.matmul(
                        sp[:],
                        w1_h[:],
                        encT[:, 4 * c : 4 * (c + 1), :],
                        start=True, stop=True,
                    )
                    nc.scalar.activation(
                        tanh_sb[:, c, :], sp[:], AF.Tanh,
                        bias=biasT_sb[:, b : b + 1],
                    )
                if cc % 2 == 1:
                    emit_logits_group(cc // 2)
                    if cc // 2 > 0:
                        emit_ctx_group(cc // 2 - 1)
                # the previous batch's softmax/ctx tail is emitted after this
                # batch's first score chunk so PE has work while the tail's
                # exp/sum chain completes on ACT/DVE
                if cc == 0 and pending_tail is not None:
                    pending_tail()
                    pending_tail = None

            def emit_tail(
                b=b, rs4=rs4, exp_full=exp_full, emit_ctx_group=emit_ctx_group
            ):
                emit_ctx_group(3)
                # ---- softmax normalization ----
                # Row 32j of exp_full holds real exp(logits); other rows
                # hold exp(0) = 1 and are excluded from the sum by `mask`.
                sm_ps = ctx_ps[:, 8 + 8 * b : 8 + 8 * b + 8]
                s_row = sm_ps[0:1, 0:4]
                nc.tensor.matmul(s_row, mask[:], rs4[:], start=True, stop=True)
                stot2 = p_small.tile([1, 2], f32, tag="stot")
                nc.vector.memset(stot2[:, 1:2], 1.0)
                nc.vector.tensor_reduce(
                    stot2[:, 0:1], s_row[:], axis=mybir.AxisListType.X,
                    op=mybir.AluOpType.add,
                )
                srec2 = p_small.tile([1, 2], f32, tag="srec")
                nc.vector.reciprocal(srec2[:], stot2[:])
                sc_ps = sm_ps[:, 4:6]
                nc.tensor.matmul(
                    sc_ps, ones_row[:], srec2[:], start=True, stop=True
                )
                scale = p_small.tile([128, 1], f32, tag="scale")
                nc.scalar.activation(scale[:], sc_ps[:, 0:1], AF.Copy)

                # normalized attention weights -> HBM (f32, exact path)
                attn_sb = p_small.tile([128, 4 * CH], f32, tag="attn")
                nc.vector.tensor_scalar_mul(attn_sb[:], exp_full[:], scale[:])
                for g in range(N_CH // 4):
                    nc.sync.dma_start(
                        attn_out.ap()[b, 2048 * g : 2048 * (g + 1)].rearrange(
                            "(j u) -> j u", u=CH
                        ),
                        attn_sb[0:97:32, CH * g : CH * (g + 1)],
                    )

                # ctx accumulated with unnormalized weights; apply 1/sum here
                nc.scalar.activation(
                    ctx_sb[:, b : b + 1], ctx_ps[:, 2 * b : 2 * b + 1], AF.Copy,
                    scale=scale[:],
                )

            pending_tail = emit_tail

        pending_tail()
        nc.sync.dma_start(ctx_out.ap().rearrange("b d -> d b"), ctx_sb[:])

    nc.compile()
    return nc


def _get_nc():
    global _BUILT
    if _BUILT is None:
        _BUILT = _build()
    return _BUILT


def kernel(encoder_outputs, decoder_hidden, W1, W2, V):
    from concourse.bass_utils import run_bass_kernel_spmd

    enc = np.ascontiguousarray(np.asarray(encoder_outputs, dtype=np.float32))
    dec = np.ascontiguousarray(np.asarray(decoder_hidden, dtype=np.float32))
    w1 = np.ascontiguousarray(np.asarray(W1, dtype=np.float32))
    w2 = np.ascontiguousarray(np.asarray(W2, dtype=np.float32))
    v = np.ascontiguousarray(np.asarray(V, dtype=np.float32))

    nc = _get_nc()
    in_maps = []
    for c in range(N_CORES):
        sl = slice(c * B_LOC, (c + 1) * B_LOC)
        in_maps.append(
            {"enc": enc[sl], "dec": dec[sl], "w1": w1, "w2": w2, "v": v}
        )
    res = run_bass_kernel_spmd(nc, in_maps, core_ids=list(range(N_CORES)))
    ctxs = np.concatenate([r["ctx_out"] for r in res.results], axis=0)
    attns = np.concatenate([r["attn_out"] for r in res.results], axis=0)
    return ctxs.astype(np.float32), attns.astype(np.float32)[:, :, None]


# revision 47
# speedup vs baseline: 1.0755x; 1.0337x over previous
"""Bahdanau-style attention layer on 8 Trainium2 NeuronCores.

Math (per batch b):
    bias  = dec[b] @ W2                              [D]
    score = tanh(enc[b] @ W1 + bias)                 [T, D]
    logit = score @ V                                [T]
    w     = softmax(logit)  (over T)                 [T]
    ctx   = sum_t w[t] * enc[b, t]                   [D]
Returns (ctx [B, D] f32, w [B, T, 1] f32).

Sharding: data-parallel over batch, 4 batches per core, W1/W2/V replicated.

Per-core kernel design (T=8192, D=128), one NeuronCore per 4 batches:
  * enc is cast f32->fp16 in-flight during the HBM load (SWDGE cast DMA)
    and kept SBUF-resident per batch in natural [t=128, chunk, d=128]
    layout. HBM is read exactly once (~16.8 MB/core, the roofline term).
  * encT (d on partitions) for the score matmul is produced by PE-mode
    transposes (fp16 stationary x identity -> fp16 PSUM), evacuated to
    SBUF by DVE copies viewed as int32 (half the element count).
  * scoreT chunk [e=128, 512] = W1^T @ encT on PE (fp16 in, f32 psum).
  * tanh(score + bias) on ACT per chunk (bias is a per-partition AP).
  * logits: V is zero-padded to 32 columns so each [32, 512] matmul fills
    a full partition group; chunk c = 4g+j lands in row 32j of PSUM bank
    g via tile_position, so one contiguous ACT exp evacuates 4 chunks
    (rows != 32j hold exp(0) = 1 and are masked out of the sum).
  * no max-subtraction in softmax: |logit| <= sum|V| ~ 8, exp is safely
    in range. exp row-sums come free via the ACT accum_out.
  * ctx accumulates with UNnormalized fp16 exp-weights (within fp16
    normal range) over 64 N=2 matmuls per batch (enc chunk stationary),
    so it pipelines without waiting for the softmax sum; 1/sum is applied
    at the [128, 1] PSUM evacuation. Weight columns [t=128, 1] come from
    PE transposes of the exp rows.
  * batch b's softmax/ctx tail is emitted after batch b+1's first score
    chunk so PE always has independent work during the reduction chain.
"""

import numpy as np

B, T, D, H = 32, 8192, 128, 128
N_CORES = 8
B_LOC = B // N_CORES          # 4 batches per core
CH = 512                      # t elements per score/logit chunk
N_CH = T // CH                # 16 chunks per batch
N_TILE = T // 128             # 64 natural [128, 128] tiles per batch

_BUILT = None


def _build():
    from contextlib import ExitStack

    import concourse.bass as bass
    import concourse.mybir as mybir
    import concourse.tile as tile
    from concourse import bacc
    from concourse.masks import make_identity

    f32 = mybir.dt.float32
    f16 = mybir.dt.float16
    AF = mybir.ActivationFunctionType

    nc = bacc.Bacc("TRN2", target_bir_lowering=False)

    enc = nc.dram_tensor("enc", [B_LOC, T, D], f32, kind="ExternalInput")
    dec = nc.dram_tensor("dec", [B_LOC, H], f32, kind="ExternalInput")
    w1 = nc.dram_tensor("w1", [D, D], f32, kind="ExternalInput")
    w2 = nc.dram_tensor("w2", [H, D], f32, kind="ExternalInput")
    vv = nc.dram_tensor("v", [D, 1], f32, kind="ExternalInput")
    ctx_out = nc.dram_tensor("ctx_out", [B_LOC, D], f32, kind="ExternalOutput")
    attn_out = nc.dram_tensor("attn_out", [B_LOC, T], f32, kind="ExternalOutput")

    with tile.TileContext(nc) as tc, ExitStack() as ctx:
        ep = ctx.enter_context  # shorthand

        # ---- pools ----
        p_const = ep(tc.tile_pool(name="const", bufs=1))
        p_enc = ep(tc.tile_pool(name="enc", bufs=3))
        p_encT = ep(tc.tile_pool(name="encT", bufs=3))
        p_tanh = ep(tc.tile_pool(name="tanh", bufs=2))
        p_small = ep(tc.tile_pool(name="small", bufs=2))
        p_ps_score = ep(tc.tile_pool(name="ps_score", bufs=2, space="PSUM"))
        p_ps_lg = ep(tc.tile_pool(name="ps_lg", bufs=1, space="PSUM"))
        p_ps_ctx = ep(tc.tile_pool(name="ps_ctx", bufs=1, space="PSUM"))
        p_ps_tr = ep(tc.tile_pool(name="ps_tr", bufs=2, space="PSUM"))

        # ---- one-time setup ----
        # Const loads go through HWDGE (f32) + DVE cast so the Pool engine's
        # SWDGE descriptor generator is free for the big enc cast-loads.
        w1_f = p_const.tile([128, 128], f32)
        nc.sync.dma_start(w1_f[:], w1.ap())
        w2_f = p_const.tile([128, 128], f32)
        nc.sync.dma_start(w2_f[:], w2.ap())
        v_f = p_const.tile([128, 1], f32)
        nc.sync.dma_start(v_f[:], vv.ap())
        decT_f = p_const.tile([128, B_LOC], f32)
        nc.sync.dma_start(decT_f[:], dec.ap().rearrange("b h -> h b"))
        w1_h = p_const.tile([128, 128], f16)
        nc.vector.tensor_copy(w1_h[:], w1_f[:])
        w2_h = p_const.tile([128, 128], f16)
        nc.vector.tensor_copy(w2_h[:], w2_f[:])
        v_h = p_const.tile([128, 1], f16)
        nc.vector.tensor_copy(v_h[:], v_f[:])
        decT_h = p_const.tile([128, B_LOC], f16)
        nc.vector.tensor_copy(decT_h[:], decT_f[:])
        ones_row = p_const.tile([1, 128], f32)
        nc.vector.memset(ones_row[:], 1.0)
        # V padded to 32 columns of zeros so the logits matmul fills a full
        # 32-partition group (only row 32j of each group is meaningful).
        v_pad = p_const.tile([128, 32], f16)
        nc.vector.memset(v_pad[:], 0.0)
        nc.vector.tensor_copy(v_pad[:, 0:1], v_h[:])
        # fp16 identity for PE-mode transposes
        ident_h = p_const.tile([128, 128], f16)
        make_identity(nc, ident_h[:])
        # mask selecting rows {0, 32, 64, 96} (where real logits live)
        mask = p_const.tile([128, 1], f32)
        nc.vector.memset(mask[:], 0.0)
        for j in range(4):
            nc.vector.memset(mask[32 * j : 32 * j + 1, :], 1.0)

        # biasT[d, b] = sum_h W2[h, d] * dec[b, h]
        biasT_ps = p_ps_score.tile([128, B_LOC], f32, tag="score")
        nc.tensor.matmul(biasT_ps[:], w2_h[:], decT_h[:], start=True, stop=True)
        biasT_sb = p_const.tile([128, B_LOC], f32)
        nc.vector.tensor_copy(biasT_sb[:], biasT_ps[:])

        # ctx uses N=2 matmuls (a garbage second column) because N=1 moving
        # APs collapse to 1-D, which the BIR verifier rejects. The same bank
        # also hosts the per-batch softmax scalars (cols 8+).
        ctx_ps = p_ps_ctx.tile([128, 2 * B_LOC + 8 * B_LOC], f32)
        ctx_sb = p_const.tile([128, B_LOC], f32)

        pending_tail = None

        for b in range(B_LOC):
            # ---- load + transpose enc (fp16) ----
            enc_nat = p_enc.tile([128, N_TILE, 128], f16, tag="enc")
            src = enc.ap()[b].rearrange("(n p) d -> p n d", p=128)
            splits = (0, 4, 8, 16, 24, 32, 40, 48, 56, 64) if b == 0 else (
                0, 8, 16, 24, 32, 40, 48, 56, 64)
            for lo, hi in zip(splits[:-1], splits[1:]):
                nc.gpsimd.dma_start(enc_nat[:, lo:hi, :], src[:, lo:hi, :])
            encT = p_encT.tile([128, N_TILE, 128], f16, tag="encT")

            # ---- transpose (PE) + scores + tanh ----
            tanh_sb = p_tanh.tile([128, N_CH, CH], f16, tag="tanh")
            i32 = mybir.dt.int32
            # Logits chunk c = 4g + j lands in row 32j of PSUM bank g (rows
            # 32j+1..32j+31 get zeros from the padded V columns), so each
            # bank is fully written and evacuates with one contiguous ACT
            # exp op. Logit groups are interleaved into the score loop as
            # soon as their tanh inputs exist.
            exp_full = p_small.tile([128, 4 * CH], f32, tag="exp")
            rs4 = p_small.tile([128, 4], f32, tag="rs4")

            w16 = p_small.tile([128, 4 * CH], f16, tag="w16")
            wT = p_small.tile([128, 16, 128], f16, tag="wT")

            def emit_logits_group(g):
                lg = p_ps_lg.tile([128, CH], f32, tag="lg")
                for j in range(4):
                    c = 4 * g + j
                    nc.tensor.matmul(
                        lg[32 * j : 32 * j + 32, :],
                        v_pad[:],
                        tanh_sb[:, c, :],
                        start=True,
                        stop=True,
                        tile_position=(0, 32 * j),
                    )
                nc.scalar.activation(
                    exp_full[:, CH * g : CH * (g + 1)], lg[:], AF.Exp,
                    accum_out=rs4[:, g : g + 1],
                )
                # Unnormalized fp16 weights (exp(logit) is within fp16 normal
                # range since |logit| <= sum|V|); 1/sum is applied at ctx evac,
                # so the ctx matmuls don't wait on the full softmax sum.
                nc.vector.tensor_copy(
                    w16[:, CH * g : CH * (g + 1)],
                    exp_full[:, CH * g : CH * (g + 1)],
                )

            def emit_ctx_group(g, b=b, w16=w16, wT=wT, enc_nat=enc_nat):
                tpw = p_ps_tr.tile([128, CH], f16, tag="tr")
                for mq in range(4):
                    m = 4 * g + mq
                    nc.tensor.transpose(
                        tpw[:, 128 * mq : 128 * mq + 128],
                        w16[:, 128 * m : 128 * (m + 1)],
                        ident_h[:],
                    )
                nc.vector.tensor_copy(
                    wT[:, 4 * g : 4 * (g + 1), :].bitcast(i32),
                    tpw[:].bitcast(i32),
                )
                for k in range(16 * g, 16 * g + 16):
                    r = 32 * ((k // 4) % 4)
                    nc.tensor.matmul(
                        ctx_ps[:, 2 * b : 2 * b + 2],
                        enc_nat[:, k, :],
                        wT[:, 4 * (k // 16) + k % 4, r : r + 2],
                        start=(k == 0),
                        stop=(k == N_TILE - 1),
                        skip_group_check=True,
                    )

            for cc in range(N_CH // 2):
                # one full PSUM bank holds all 8 transposes of this cc-pair
                tp = p_ps_tr.tile([128, 2 * CH], f16, tag="tr")
                for q in range(8):
                    k = 8 * cc + q
                    nc.tensor.transpose(
                        tp[:, 128 * q : 128 * (q + 1)],
                        enc_nat[:, k, :],
                        ident_h[:],
                    )
                nc.vector.tensor_copy(
                    encT[:, 8 * cc : 8 * (cc + 1), :].bitcast(i32),
                    tp[:].bitcast(i32),
                )
                sp = p_ps_score.tile([128, 2 * CH], f32, tag="score")
                for c in (2 * cc, 2 * cc + 1):
                    nc.tensor.matmul(
                        sp[:, CH * (c - 2 * cc) : CH * (c - 2 * cc + 1)],
                        w1_h[:],
                        encT[:, 4 * c : 4 * (c + 1), :],
                        start=True, stop=True,
                    )
                nc.scalar.activation(
                    tanh_sb[:, 2 * cc : 2 * cc + 2, :], sp[:], AF.Tanh,
                    bias=biasT_sb[:, b : b + 1],
                )
                if cc % 2 == 1:
                    emit_logits_group(cc // 2)
                    if cc // 2 > 0:
                        emit_ctx_group(cc // 2 - 1)
                # the previous batch's softmax/ctx tail is emitted after this
                # batch's first score chunk so PE has work while the tail's
                # exp/sum chain completes on ACT/DVE
                if cc == 0 and pending_tail is not None:
                    pending_tail()
                    pending_tail = None

            def emit_tail(
                b=b, rs4=rs4, exp_full=exp_full, emit_ctx_group=emit_ctx_group
            ):
                emit_ctx_group(3)
                # ---- softmax normalization ----
                # Row 32j of exp_full holds real exp(logits); other rows
                # hold exp(0) = 1 and are excluded from the sum by `mask`.
                sm_ps = ctx_ps[:, 8 + 8 * b : 8 + 8 * b + 8]
                s_row = sm_ps[0:1, 0:4]
                nc.tensor.matmul(s_row, mask[:], rs4[:], start=True, stop=True)
                stot2 = p_small.tile([1, 2], f32, tag="stot")
                nc.vector.memset(stot2[:, 1:2], 1.0)
                nc.vector.tensor_reduce(
                    stot2[:, 0:1], s_row[:], axis=mybir.AxisListType.X,
                    op=mybir.AluOpType.add,
                )
                srec2 = p_small.tile([1, 2], f32, tag="srec")
                nc.vector.reciprocal(srec2[:], stot2[:])
                sc_ps = sm_ps[:, 4:6]
                nc.tensor.matmul(
                    sc_ps, ones_row[:], srec2[:], start=True, stop=True
                )
                scale = p_small.tile([128, 1], f32, tag="scale")
                nc.vector.tensor_copy(scale[:], sc_ps[:, 0:1])

                # normalized attention weights -> HBM (f32, exact path)
                attn_sb = p_small.tile([128, 4 * CH], f32, tag="attn")
                nc.vector.tensor_scalar_mul(attn_sb[:], exp_full[:], scale[:])
                for g in range(N_CH // 4):
                    nc.sync.dma_start(
                        attn_out.ap()[b, 2048 * g : 2048 * (g + 1)].rearrange(
                            "(j u) -> j u", u=CH
                        ),
                        attn_sb[0:97:32, CH * g : CH * (g + 1)],
                    )

                # ctx accumulated with unnormalized weights; apply 1/sum here
                nc.vector.tensor_scalar_mul(
                    ctx_sb[:, b : b + 1], ctx_ps[:, 2 * b : 2 * b + 1], scale[:]
                )

            pending_tail = emit_tail

        pending_tail()
        nc.sync.dma_start(ctx_out.ap().rearrange("b d -> d b"), ctx_sb[:])

    nc.compile()
    return nc


def _get_nc():
    global _BUILT
    if _BUILT is None:
        _BUILT = _build()
    return _BUILT


def kernel(encoder_outputs, decoder_hidden, W1, W2, V):
    from concourse.bass_utils import run_bass_kernel_spmd

    enc = np.ascontiguousarray(np.asarray(encoder_outputs, dtype=np.float32))
    dec = np.ascontiguousarray(np.asarray(decoder_hidden, dtype=np.float32))
    w1 = np.ascontiguousarray(np.asarray(W1, dtype=np.float32))
    w2 = np.ascontiguousarray(np.asarray(W2, dtype=np.float32))
    v = np.ascontiguousarray(np.asarray(V, dtype=np.float32))

    nc = _get_nc()
    in_maps = []
    for c in range(N_CORES):
        sl = slice(c * B_LOC, (c + 1) * B_LOC)
        in_maps.append(
            {"enc": enc[sl], "dec": dec[sl], "w1": w1, "w2": w2, "v": v}
        )
    res = run_bass_kernel_spmd(nc, in_maps, core_ids=list(range(N_CORES)))
    ctxs = np.concatenate([r["ctx_out"] for r in res.results], axis=0)
    attns = np.concatenate([r["attn_out"] for r in res.results], axis=0)
    return ctxs.astype(np.float32), attns.astype(np.float32)[:, :, None]


# revision 49
# speedup vs baseline: 1.0847x; 1.0085x over previous
"""Bahdanau-style attention layer on 8 Trainium2 NeuronCores.

Math (per batch b):
    bias  = dec[b] @ W2                              [D]
    score = tanh(enc[b] @ W1 + bias)                 [T, D]
    logit = score @ V                                [T]
    w     = softmax(logit)  (over T)                 [T]
    ctx   = sum_t w[t] * enc[b, t]                   [D]
Returns (ctx [B, D] f32, w [B, T, 1] f32).

Sharding: data-parallel over batch, 4 batches per core, W1/W2/V replicated.

Per-core kernel design (T=8192, D=128), one NeuronCore per 4 batches:
  * enc is cast f32->fp16 in-flight during the HBM load (SWDGE cast DMA)
    and kept SBUF-resident per batch in natural [t=128, chunk, d=128]
    layout. HBM is read exactly once (~16.8 MB/core, the roofline term).
  * encT (d on partitions) for the score matmul is produced by PE-mode
    transposes (fp16 stationary x identity -> fp16 PSUM), evacuated to
    SBUF by DVE copies viewed as int32 (half the element count).
  * scoreT chunk [e=128, 512] = W1^T @ encT on PE (fp16 in, f32 psum).
  * tanh(score + bias) on ACT per chunk (bias is a per-partition AP).
  * logits: V is zero-padded to 32 columns so each [32, 512] matmul fills
    a full partition group; chunk c = 4g+j lands in row 32j of PSUM bank
    g via tile_position, so one contiguous ACT exp evacuates 4 chunks
    (rows != 32j hold exp(0) = 1 and are masked out of the sum).
  * no max-subtraction in softmax: |logit| <= sum|V| ~ 8, exp is safely
    in range. exp row-sums come free via the ACT accum_out.
  * ctx accumulates with UNnormalized fp16 exp-weights (within fp16
    normal range) over 64 N=2 matmuls per batch (enc chunk stationary),
    so it pipelines without waiting for the softmax sum; 1/sum is applied
    at the [128, 1] PSUM evacuation. Weight columns [t=128, 1] come from
    PE transposes of the exp rows.
  * batch b's softmax/ctx tail is emitted after batch b+1's first score
    chunk so PE always has independent work during the reduction chain.
"""

import numpy as np

B, T, D, H = 32, 8192, 128, 128
N_CORES = 8
B_LOC = B // N_CORES          # 4 batches per core
CH = 512                      # t elements per score/logit chunk
N_CH = T // CH                # 16 chunks per batch
N_TILE = T // 128             # 64 natural [128, 128] tiles per batch

_BUILT = None


def _build():
    from contextlib import ExitStack

    import concourse.bass as bass
    import concourse.mybir as mybir
    import concourse.tile as tile
    from concourse import bacc
    from concourse.masks import make_identity

    f32 = mybir.dt.float32
    f16 = mybir.dt.float16
    AF = mybir.ActivationFunctionType

    nc = bacc.Bacc("TRN2", target_bir_lowering=False)

    enc = nc.dram_tensor("enc", [B_LOC, T, D], f32, kind="ExternalInput")
    dec = nc.dram_tensor("dec", [B_LOC, H], f32, kind="ExternalInput")
    w1 = nc.dram_tensor("w1", [D, D], f32, kind="ExternalInput")
    w2 = nc.dram_tensor("w2", [H, D], f32, kind="ExternalInput")
    vv = nc.dram_tensor("v", [D, 1], f32, kind="ExternalInput")
    ctx_out = nc.dram_tensor("ctx_out", [B_LOC, D], f32, kind="ExternalOutput")
    attn_out = nc.dram_tensor("attn_out", [B_LOC, T], f32, kind="ExternalOutput")

    with tile.TileContext(nc) as tc, ExitStack() as ctx:
        ep = ctx.enter_context  # shorthand

        # ---- pools ----
        p_const = ep(tc.tile_pool(name="const", bufs=1))
        p_enc = ep(tc.tile_pool(name="enc", bufs=3))
        p_encT = ep(tc.tile_pool(name="encT", bufs=3))
        p_tanh = ep(tc.tile_pool(name="tanh", bufs=2))
        p_small = ep(tc.tile_pool(name="small", bufs=2))
        p_ps_score = ep(tc.tile_pool(name="ps_score", bufs=4, space="PSUM"))
        p_ps_lg = ep(tc.tile_pool(name="ps_lg", bufs=1, space="PSUM"))
        p_ps_ctx = ep(tc.tile_pool(name="ps_ctx", bufs=1, space="PSUM"))
        p_ps_tr = ep(tc.tile_pool(name="ps_tr", bufs=2, space="PSUM"))

        # ---- one-time setup ----
        # Const loads go through HWDGE (f32) + DVE cast so the Pool engine's
        # SWDGE descriptor generator is free for the big enc cast-loads.
        w1_f = p_const.tile([128, 128], f32)
        nc.sync.dma_start(w1_f[:], w1.ap())
        w2_f = p_const.tile([128, 128], f32)
        nc.sync.dma_start(w2_f[:], w2.ap())
        v_f = p_const.tile([128, 1], f32)
        nc.sync.dma_start(v_f[:], vv.ap())
        decT_f = p_const.tile([128, B_LOC], f32)
        nc.sync.dma_start(decT_f[:], dec.ap().rearrange("b h -> h b"))
        w1_h = p_const.tile([128, 128], f16)
        nc.vector.tensor_copy(w1_h[:], w1_f[:])
        w2_h = p_const.tile([128, 128], f16)
        nc.vector.tensor_copy(w2_h[:], w2_f[:])
        v_h = p_const.tile([128, 1], f16)
        nc.vector.tensor_copy(v_h[:], v_f[:])
        decT_h = p_const.tile([128, B_LOC], f16)
        nc.vector.tensor_copy(decT_h[:], decT_f[:])
        ones_row = p_const.tile([1, 128], f32)
        nc.vector.memset(ones_row[:], 1.0)
        # V padded to 32 columns of zeros so the logits matmul fills a full
        # 32-partition group (only row 32j of each group is meaningful).
        v_pad = p_const.tile([128, 32], f16)
        nc.vector.memset(v_pad[:], 0.0)
        nc.vector.tensor_copy(v_pad[:, 0:1], v_h[:])
        # fp16 identity for PE-mode transposes
        ident_h = p_const.tile([128, 128], f16)
        make_identity(nc, ident_h[:])
        # mask selecting rows {0, 32, 64, 96} (where real logits live)
        mask = p_const.tile([128, 1], f32)
        nc.vector.memset(mask[:], 0.0)
        for j in range(4):
            nc.vector.memset(mask[32 * j : 32 * j + 1, :], 1.0)

        # biasT[d, b] = sum_h W2[h, d] * dec[b, h]
        biasT_ps = p_ps_score.tile([128, B_LOC], f32, tag="score")
        nc.tensor.matmul(biasT_ps[:], w2_h[:], decT_h[:], start=True, stop=True)
        biasT_sb = p_const.tile([128, B_LOC], f32)
        nc.vector.tensor_copy(biasT_sb[:], biasT_ps[:])

        # ctx uses N=2 matmuls (a garbage second column) because N=1 moving
        # APs collapse to 1-D, which the BIR verifier rejects. The same bank
        # also hosts the per-batch softmax scalars (cols 8+).
        ctx_ps = p_ps_ctx.tile([128, 2 * B_LOC + 8 * B_LOC], f32)
        ctx_sb = p_const.tile([128, B_LOC], f32)

        pending_tail = None

        for b in range(B_LOC):
            # ---- load + transpose enc (fp16) ----
            enc_nat = p_enc.tile([128, N_TILE, 128], f16, tag="enc")
            src = enc.ap()[b].rearrange("(n p) d -> p n d", p=128)
            if b == 0:
                # HWDGE starts ~2us before the first SWDGE descriptors are
                # ready; bootstrap the first tiles through it (f32 + DVE cast)
                stage0 = p_const.tile([128, 4, 128], f32)
                nc.sync.dma_start(stage0[:], src[:, 0:4, :])
                nc.vector.tensor_copy(enc_nat[:, 0:4, :], stage0[:])
                splits = (4, 8, 16, 24, 32, 40, 48, 56, 64)
            else:
                splits = (0, 8, 16, 24, 32, 40, 48, 56, 64)
            for lo, hi in zip(splits[:-1], splits[1:]):
                nc.gpsimd.dma_start(enc_nat[:, lo:hi, :], src[:, lo:hi, :])
            encT = p_encT.tile([128, N_TILE, 128], f16, tag="encT")

            # ---- transpose (PE) + scores + tanh ----
            tanh_sb = p_tanh.tile([128, N_CH, CH], f16, tag="tanh")
            i32 = mybir.dt.int32
            # Logits chunk c = 4g + j lands in row 32j of PSUM bank g (rows
            # 32j+1..32j+31 get zeros from the padded V columns), so each
            # bank is fully written and evacuates with one contiguous ACT
            # exp op. Logit groups are interleaved into the score loop as
            # soon as their tanh inputs exist.
            exp_full = p_small.tile([128, 4 * CH], f32, tag="exp")
            rs4 = p_small.tile([128, 4], f32, tag="rs4")

            w16 = p_small.tile([128, 4 * CH], f16, tag="w16")
            wT = p_small.tile([128, 16, 128], f16, tag="wT")

            def emit_logits_group(g):
                lg = p_ps_lg.tile([128, CH], f32, tag="lg")
                for j in range(4):
                    c = 4 * g + j
                    nc.tensor.matmul(
                        lg[32 * j : 32 * j + 32, :],
                        v_pad[:],
                        tanh_sb[:, c, :],
                        start=True,
                        stop=True,
                        tile_position=(0, 32 * j),
                    )
                nc.scalar.activation(
                    exp_full[:, CH * g : CH * (g + 1)], lg[:], AF.Exp,
                    accum_out=rs4[:, g : g + 1],
                )
                # Unnormalized fp16 weights (exp(logit) is within fp16 normal
                # range since |logit| <= sum|V|); 1/sum is applied at ctx evac,
                # so the ctx matmuls don't wait on the full softmax sum.
                nc.vector.tensor_copy(
                    w16[:, CH * g : CH * (g + 1)],
                    exp_full[:, CH * g : CH * (g + 1)],
                )

            def emit_ctx_group(g, b=b, w16=w16, wT=wT, enc_nat=enc_nat):
                tpw = p_ps_tr.tile([128, CH], f16, tag="tr")
                for mq in range(4):
                    m = 4 * g + mq
                    nc.tensor.transpose(
                        tpw[:, 128 * mq : 128 * mq + 128],
                        w16[:, 128 * m : 128 * (m + 1)],
                        ident_h[:],
                    )
                nc.vector.tensor_copy(
                    wT[:, 4 * g : 4 * (g + 1), :].bitcast(i32),
                    tpw[:].bitcast(i32),
                )
                for k in range(16 * g, 16 * g + 16):
                    r = 32 * ((k // 4) % 4)
                    nc.tensor.matmul(
                        ctx_ps[:, 2 * b : 2 * b + 2],
                        enc_nat[:, k, :],
                        wT[:, 4 * (k // 16) + k % 4, r : r + 2],
                        start=(k == 0),
                        stop=(k == N_TILE - 1),
                        skip_group_check=True,
                    )

            for cc in range(N_CH // 2):
                # one full PSUM bank holds all 8 transposes of this cc-pair
                tp = p_ps_tr.tile([128, 2 * CH], f16, tag="tr")
                for q in range(8):
                    k = 8 * cc + q
                    nc.tensor.transpose(
                        tp[:, 128 * q : 128 * (q + 1)],
                        enc_nat[:, k, :],
                        ident_h[:],
                    )
                nc.vector.tensor_copy(
                    encT[:, 8 * cc : 8 * (cc + 1), :].bitcast(i32),
                    tp[:].bitcast(i32),
                )
                for c in (2 * cc, 2 * cc + 1):
                    sp = p_ps_score.tile([128, CH], f32, tag="score")
                    nc.tensor.matmul(
                        sp[:],
                        w1_h[:],
                        encT[:, 4 * c : 4 * (c + 1), :],
                        start=True, stop=True,
                    )
                    nc.scalar.activation(
                        tanh_sb[:, c, :], sp[:], AF.Tanh,
                        bias=biasT_sb[:, b : b + 1],
                    )
                if cc % 2 == 1:
                    emit_logits_group(cc // 2)
                    if cc // 2 > 0:
                        emit_ctx_group(cc // 2 - 1)
                # the previous batch's softmax/ctx tail is emitted after this
                # batch's first score chunk so PE has work while the tail's
                # exp/sum chain completes on ACT/DVE
                if cc == 0 and pending_tail is not None:
                    pending_tail()
                    pending_tail = None

            def emit_tail(
                b=b, rs4=rs4, exp_full=exp_full, emit_ctx_group=emit_ctx_group
            ):
                emit_ctx_group(3)
                # ---- softmax normalization ----
                # Row 32j of exp_full holds real exp(logits); other rows
                # hold exp(0) = 1 and are excluded from the sum by `mask`.
                sm_ps = ctx_ps[:, 8 + 8 * b : 8 + 8 * b + 8]
                s_row = sm_ps[0:1, 0:4]
                nc.tensor.matmul(s_row, mask[:], rs4[:], start=True, stop=True)
                stot2 = p_small.tile([1, 2], f32, tag="stot")
                nc.vector.memset(stot2[:, 1:2], 1.0)
                nc.vector.tensor_reduce(
                    stot2[:, 0:1], s_row[:], axis=mybir.AxisListType.X,
                    op=mybir.AluOpType.add,
                )
                srec2 = p_small.tile([1, 2], f32, tag="srec")
                nc.vector.reciprocal(srec2[:], stot2[:])
                sc_ps = sm_ps[:, 4:6]
                nc.tensor.matmul(
                    sc_ps, ones_row[:], srec2[:], start=True, stop=True
                )
                scale = p_small.tile([128, 1], f32, tag="scale")
                nc.vector.tensor_copy(scale[:], sc_ps[:, 0:1])

                # normalized attention weights -> HBM (f32, exact path)
                attn_sb = p_small.tile([128, 4 * CH], f32, tag="attn")
                nc.vector.tensor_scalar_mul(attn_sb[:], exp_full[:], scale[:])
                for g in range(N_CH // 4):
                    nc.sync.dma_start(
                        attn_out.ap()[b, 2048 * g : 2048 * (g + 1)].rearrange(
                            "(j u) -> j u", u=CH
                        ),
                        attn_sb[0:97:32, CH * g : CH * (g + 1)],
                    )

                # ctx accumulated with unnormalized weights; apply 1/sum here
                nc.vector.tensor_scalar_mul(
                    ctx_sb[:, b : b + 1], ctx_ps[:, 2 * b : 2 * b + 1], scale[:]
                )

            pending_tail = emit_tail

        pending_tail()
        nc.sync.dma_start(ctx_out.ap().rearrange("b d -> d b"), ctx_sb[:])

    nc.compile()
    return nc


def _get_nc():
    global _BUILT
    if _BUILT is None:
        _BUILT = _build()
    return _BUILT


def kernel(encoder_outputs, decoder_hidden, W1, W2, V):
    from concourse.bass_utils import run_bass_kernel_spmd

    enc = np.ascontiguousarray(np.asarray(encoder_outputs, dtype=np.float32))
    dec = np.ascontiguousarray(np.asarray(decoder_hidden, dtype=np.float32))
    w1 = np.ascontiguousarray(np.asarray(W1, dtype=np.float32))
    w2 = np.ascontiguousarray(np.asarray(W2, dtype=np.float32))
    v = np.ascontiguousarray(np.asarray(V, dtype=np.float32))

    nc = _get_nc()
    in_maps = []
    for c in range(N_CORES):
        sl = slice(c * B_LOC, (c + 1) * B_LOC)
        in_maps.append(
            {"enc": enc[sl], "dec": dec[sl], "w1": w1, "w2": w2, "v": v}
        )
    res = run_bass_kernel_spmd(nc, in_maps, core_ids=list(range(N_CORES)))
    ctxs = np.concatenate([r["ctx_out"] for r in res.results], axis=0)
    attns = np.concatenate([r["attn_out"] for r in res.results], axis=0)
    return ctxs.astype(np.float32), attns.astype(np.float32)[:, :, None]


# revision 51
# speedup vs baseline: 1.2749x; 1.1754x over previous
"""Bahdanau-style attention layer on 8 Trainium2 NeuronCores.

Math (per batch b):
    bias  = dec[b] @ W2                              [D]
    score = tanh(enc[b] @ W1 + bias)                 [T, D]
    logit = score @ V                                [T]
    w     = softmax(logit)  (over T)                 [T]
    ctx   = sum_t w[t] * enc[b, t]                   [D]
Returns (ctx [B, D] f32, w [B, T, 1] f32).

Sharding: data-parallel over batch, 4 batches per core, W1/W2/V replicated.

Per-core kernel design (T=8192, D=128), one NeuronCore per 4 batches:
  * enc is cast f32->fp16 in-flight during the HBM load (SWDGE cast DMA)
    and kept SBUF-resident per batch in natural [t=128, chunk, d=128]
    layout. HBM is read exactly once (~16.8 MB/core, the roofline term).
  * encT (d on partitions) for the score matmul is produced by PE-mode
    transposes (fp16 stationary x identity -> fp16 PSUM), evacuated to
    SBUF by DVE copies viewed as int32 (half the element count).
  * scoreT chunk [e=128, 512] = W1^T @ encT on PE (fp16 in, f32 psum).
  * tanh(score + bias) on ACT per chunk (bias is a per-partition AP).
  * logits: V is zero-padded to 32 columns so each [32, 512] matmul fills
    a full partition group; chunk c = 4g+j lands in row 32j of PSUM bank
    g via tile_position, so one contiguous ACT exp evacuates 4 chunks
    (rows != 32j hold exp(0) = 1 and are masked out of the sum).
  * no max-subtraction in softmax: |logit| <= sum|V| ~ 8, exp is safely
    in range. exp row-sums come free via the ACT accum_out.
  * ctx accumulates with UNnormalized fp16 exp-weights (within fp16
    normal range) over 64 N=2 matmuls per batch (enc chunk stationary),
    so it pipelines without waiting for the softmax sum; 1/sum is applied
    at the [128, 1] PSUM evacuation. Weight columns [t=128, 1] come from
    PE transposes of the exp rows.
  * batch b's softmax/ctx tail is emitted after batch b+1's first score
    chunk so PE always has independent work during the reduction chain.
"""

import numpy as np

B, T, D, H = 32, 8192, 128, 128
N_CORES = 8
B_LOC = B // N_CORES          # 4 batches per core
CH = 512                      # t elements per score/logit chunk
N_CH = T // CH                # 16 chunks per batch
N_TILE = T // 128             # 64 natural [128, 128] tiles per batch

_BUILT = None


def _build():
    from contextlib import ExitStack

    import concourse.bass as bass
    import concourse.mybir as mybir
    import concourse.tile as tile
    from concourse import bacc
    from concourse.masks import make_identity

    f32 = mybir.dt.float32
    f16 = mybir.dt.float16
    AF = mybir.ActivationFunctionType

    nc = bacc.Bacc("TRN2", target_bir_lowering=False)

    enc = nc.dram_tensor("enc", [B_LOC, T, D], f32, kind="ExternalInput")
    dec = nc.dram_tensor("dec", [B_LOC, H], f32, kind="ExternalInput")
    w1 = nc.dram_tensor("w1", [D, D], f32, kind="ExternalInput")
    w2 = nc.dram_tensor("w2", [H, D], f32, kind="ExternalInput")
    vv = nc.dram_tensor("v", [D, 1], f32, kind="ExternalInput")
    ctx_out = nc.dram_tensor("ctx_out", [B_LOC, D], f32, kind="ExternalOutput")
    attn_out = nc.dram_tensor("attn_out", [B_LOC, T], f32, kind="ExternalOutput")

    with tile.TileContext(nc) as tc, ExitStack() as ctx:
        ep = ctx.enter_context  # shorthand

        # ---- pools ----
        p_const = ep(tc.tile_pool(name="const", bufs=1))
        p_enc = ep(tc.tile_pool(name="enc", bufs=3))
        p_encT = ep(tc.tile_pool(name="encT", bufs=3))
        p_tanh = ep(tc.tile_pool(name="tanh", bufs=2))
        p_small = ep(tc.tile_pool(name="small", bufs=2))
        p_ps_score = ep(tc.tile_pool(name="ps_score", bufs=4, space="PSUM"))
        p_ps_lg = ep(tc.tile_pool(name="ps_lg", bufs=1, space="PSUM"))
        p_ps_ctx = ep(tc.tile_pool(name="ps_ctx", bufs=1, space="PSUM"))
        p_ps_tr = ep(tc.tile_pool(name="ps_tr", bufs=2, space="PSUM"))

        # ---- one-time setup ----
        # Const loads go through HWDGE (f32) + DVE cast so the Pool engine's
        # SWDGE descriptor generator is free for the big enc cast-loads.
        w1_f = p_const.tile([128, 128], f32)
        nc.sync.dma_start(w1_f[:], w1.ap())
        w2_f = p_const.tile([128, 128], f32)
        nc.sync.dma_start(w2_f[:], w2.ap())
        v_f = p_const.tile([128, 1], f32)
        nc.sync.dma_start(v_f[:], vv.ap())
        decT_f = p_const.tile([128, B_LOC], f32)
        nc.sync.dma_start(decT_f[:], dec.ap().rearrange("b h -> h b"))
        w1_h = p_const.tile([128, 128], f16)
        nc.vector.tensor_copy(w1_h[:], w1_f[:])
        w2_h = p_const.tile([128, 128], f16)
        nc.vector.tensor_copy(w2_h[:], w2_f[:])
        v_h = p_const.tile([128, 1], f16)
        nc.vector.tensor_copy(v_h[:], v_f[:])
        decT_h = p_const.tile([128, B_LOC], f16)
        nc.vector.tensor_copy(decT_h[:], decT_f[:])
        ones_row = p_const.tile([1, 128], f32)
        nc.vector.memset(ones_row[:], 1.0)
        # V plus a zero column (N=1 moving APs collapse to 1-D -> illegal)
        v_pad = p_const.tile([128, 2], f16)
        nc.vector.memset(v_pad[:], 0.0)
        nc.vector.tensor_copy(v_pad[:, 0:1], v_h[:])
        # fp16 identity for PE-mode transposes
        ident_h = p_const.tile([128, 128], f16)
        make_identity(nc, ident_h[:])
        ones_col = p_const.tile([128, 1], f32)
        nc.vector.memset(ones_col[:], 1.0)

        # biasT[d, b] = sum_h W2[h, d] * dec[b, h]
        biasT_ps = p_ps_score.tile([128, B_LOC], f32, tag="score")
        nc.tensor.matmul(biasT_ps[:], w2_h[:], decT_h[:], start=True, stop=True)
        biasT_sb = p_const.tile([128, B_LOC], f32)
        nc.vector.tensor_copy(biasT_sb[:], biasT_ps[:])

        # ctx uses N=2 matmuls (a garbage second column) because N=1 moving
        # APs collapse to 1-D, which the BIR verifier rejects. The same bank
        # also hosts the per-batch softmax scalars (cols 8+).
        ctx_ps = p_ps_ctx.tile([128, 2 * B_LOC + 8 * B_LOC], f32)
        ctx_sb = p_const.tile([128, B_LOC], f32)

        pending_tail = None

        for b in range(B_LOC):
            # ---- load + transpose enc (fp16) ----
            enc_nat = p_enc.tile([128, N_TILE, 128], f16, tag="enc")
            src = enc.ap()[b].rearrange("(n p) d -> p n d", p=128)
            if b == 0:
                # HWDGE starts ~2us before the first SWDGE descriptors are
                # ready; bootstrap the first tiles through it (f32 + DVE cast)
                stage0 = p_const.tile([128, 4, 128], f32)
                nc.sync.dma_start(stage0[:], src[:, 0:4, :])
                nc.vector.tensor_copy(enc_nat[:, 0:4, :], stage0[:])
                splits = (4, 8, 16, 24, 32, 40, 48, 56, 64)
            else:
                splits = (0, 8, 16, 24, 32, 40, 48, 56, 64)
            for lo, hi in zip(splits[:-1], splits[1:]):
                nc.gpsimd.dma_start(enc_nat[:, lo:hi, :], src[:, lo:hi, :])
            encT = p_encT.tile([128, N_TILE, 128], f16, tag="encT")

            # ---- transpose (PE) + scores + tanh + dense logits ----
            tanh_sb = p_tanh.tile([128, N_CH, CH], f16, tag="tanh")
            i32 = mybir.dt.int32
            # Dense logits: each 128-t slice of tanh is the STATIONARY
            # operand against [V | 0], so logits land t-on-partitions:
            # lgT[p, 2k] = logit[t = 128k + p]. One [128, 64] exp per batch,
            # and ctx weight columns need no transpose at all.
            lgT = p_ps_lg.tile([128, 2 * N_TILE], f32, tag="lg")

            for cc in range(N_CH // 2):
                # one full PSUM bank holds all 8 transposes of this cc-pair
                tp = p_ps_tr.tile([128, 2 * CH], f16, tag="tr")
                for q in range(8):
                    k = 8 * cc + q
                    nc.tensor.transpose(
                        tp[:, 128 * q : 128 * (q + 1)],
                        enc_nat[:, k, :],
                        ident_h[:],
                    )
                nc.vector.tensor_copy(
                    encT[:, 8 * cc : 8 * (cc + 1), :].bitcast(i32),
                    tp[:].bitcast(i32),
                )
                for c in (2 * cc, 2 * cc + 1):
                    sp = p_ps_score.tile([128, CH], f32, tag="score")
                    nc.tensor.matmul(
                        sp[:],
                        w1_h[:],
                        encT[:, 4 * c : 4 * (c + 1), :],
                        start=True, stop=True,
                    )
                    nc.scalar.activation(
                        tanh_sb[:, c, :], sp[:], AF.Tanh,
                        bias=biasT_sb[:, b : b + 1],
                    )
                    for s in range(4):
                        k = 4 * c + s
                        nc.tensor.matmul(
                            lgT[:, 2 * k : 2 * k + 2],
                            tanh_sb[:, c, 128 * s : 128 * (s + 1)],
                            v_pad[:],
                            start=True, stop=True,
                        )
                # the previous batch's softmax/ctx tail is emitted after this
                # batch's first score chunk so PE has work while the tail's
                # exp/sum chain completes on ACT/DVE
                if cc == 0 and pending_tail is not None:
                    pending_tail()
                    pending_tail = None

            def emit_tail(b=b, lgT=lgT, enc_nat=enc_nat):
                # exp of all 8192 logits in one op, row sums via accum_out
                rs2 = p_small.tile([128, 2], f32, tag="rs2")
                nc.vector.memset(rs2[:], 0.0)
                exp_d = p_small.tile([128, N_TILE], f32, tag="exp")
                nc.scalar.activation(
                    exp_d[:], lgT[:, 0 : 2 * N_TILE : 2], AF.Exp,
                    accum_out=rs2[:, 0:1],
                )
                # Unnormalized fp16 weights (exp(logit) is within fp16 normal
                # range since |logit| <= sum|V|); 1/sum is applied at the ctx
                # evacuation. Column 64 is zero padding for the N=2 matmuls.
                w16d = p_small.tile([128, N_TILE + 2], f16, tag="w16")
                nc.vector.memset(w16d[:, N_TILE : N_TILE + 2], 0.0)
                nc.vector.tensor_copy(w16d[:, 0:N_TILE], exp_d[:])
                for k in range(N_TILE):
                    nc.tensor.matmul(
                        ctx_ps[:, 2 * b : 2 * b + 2],
                        enc_nat[:, k, :],
                        w16d[:, k : k + 2],
                        start=(k == 0),
                        stop=(k == N_TILE - 1),
                        skip_group_check=True,
                    )

                # ---- softmax normalization ----
                sm_ps = ctx_ps[:, 8 + 8 * b : 8 + 8 * b + 8]
                s_row = sm_ps[0:1, 0:2]
                nc.tensor.matmul(s_row, ones_col[:], rs2[:], start=True, stop=True)
                stot2 = p_small.tile([1, 2], f32, tag="stot")
                nc.vector.memset(stot2[:, 1:2], 1.0)
                nc.vector.tensor_reduce(
                    stot2[:, 0:1], s_row[:], axis=mybir.AxisListType.X,
                    op=mybir.AluOpType.add,
                )
                srec2 = p_small.tile([1, 2], f32, tag="srec")
                nc.vector.reciprocal(srec2[:], stot2[:])
                sc_ps = sm_ps[:, 4:6]
                nc.tensor.matmul(
                    sc_ps, ones_row[:], srec2[:], start=True, stop=True
                )
                scale = p_small.tile([128, 1], f32, tag="scale")
                nc.vector.tensor_copy(scale[:], sc_ps[:, 0:1])

                # attention weights: PE-transpose to t-major rows, normalize
                # by 1/sum (per-partition scale AP) during the f32 evacuation
                attnT = p_ps_tr.tile([128, 128], f16, tag="tr")
                nc.tensor.transpose(
                    attnT[0:N_TILE, :], w16d[:, 0:N_TILE], ident_h[:]
                )
                attn_rows = p_small.tile([N_TILE, 128], f32, tag="attn")
                nc.scalar.activation(
                    attn_rows[:], attnT[0:N_TILE, :], AF.Copy,
                    scale=scale[0:N_TILE, :],
                )
                nc.sync.dma_start(
                    attn_out.ap()[b].rearrange("(k p) -> k p", p=128),
                    attn_rows[:],
                )

                # ctx accumulated with unnormalized weights; apply 1/sum here
                nc.vector.tensor_scalar_mul(
                    ctx_sb[:, b : b + 1], ctx_ps[:, 2 * b : 2 * b + 1], scale[:]
                )

            pending_tail = emit_tail

        pending_tail()
        nc.sync.dma_start(ctx_out.ap().rearrange("b d -> d b"), ctx_sb[:])

    nc.compile()
    return nc


def _get_nc():
    global _BUILT
    if _BUILT is None:
        _BUILT = _build()
    return _BUILT


def kernel(encoder_outputs, decoder_hidden, W1, W2, V):
    from concourse.bass_utils import run_bass_kernel_spmd

    enc = np.ascontiguousarray(np.asarray(encoder_outputs, dtype=np.float32))
    dec = np.ascontiguousarray(np.asarray(decoder_hidden, dtype=np.float32))
    w1 = np.ascontiguousarray(np.asarray(W1, dtype=np.float32))
    w2 = np.ascontiguousarray(np.asarray(W2, dtype=np.float32))
    v = np.ascontiguousarray(np.asarray(V, dtype=np.float32))

    nc = _get_nc()
    in_maps = []
    for c in range(N_CORES):
        sl = slice(c * B_LOC, (c + 1) * B_LOC)
        in_maps.append(
            {"enc": enc[sl], "dec": dec[sl], "w1": w1, "w2": w2, "v": v}
        )
    res = run_bass_kernel_spmd(nc, in_maps, core_ids=list(range(N_CORES)))
    ctxs = np.concatenate([r["ctx_out"] for r in res.results], axis=0)
    attns = np.concatenate([r["attn_out"] for r in res.results], axis=0)
    return ctxs.astype(np.float32), attns.astype(np.float32)[:, :, None]
